# revision 3
# baseline (speedup 1.0000x reference)
"""Attention kernel: int8-quantized KV-cache attention with fused int8 QKV/WO.

Sharding strategy (tensor parallel over heads, 8 cores):
  - q heads (32) -> 4 per core; kv heads (8) -> 1 per core
  - cache_k/cache_v/wqkv sliced by head; wo row-parallel; x replicated
  - final output reduced across cores

Shapes (hardcoded per problem spec):
  B=4, S=16, L=8192, D=4096, H=32, HKV=8, HD=128
"""
import numpy as np

B, S, L, D, H, HKV, HD = 4, 16, 8192, 4096, 32, 8, 128
Q_SIZE = H * HD
KV_SIZE = HKV * HD
N_CORES = 8
G = H // HKV  # q heads per kv head


def _rope(x, cos, sin):
    # x: [B, S, h, HD]; cos/sin: [S, HD//2]; interleaved-pair rotation
    xr = x.reshape(*x.shape[:-1], HD // 2, 2)
    x0, x1 = xr[..., 0], xr[..., 1]
    c = cos[None, :, None, :]
    s = sin[None, :, None, :]
    o0 = x0 * c - x1 * s
    o1 = x0 * s + x1 * c
    return np.stack([o0, o1], axis=-1).reshape(x.shape).astype(np.float32)


def _softmax(x, axis=-1):
    m = np.max(x, axis=axis, keepdims=True)
    e = np.exp(x - m)
    return e / np.sum(e, axis=axis, keepdims=True)


def _core_attention(q, keys, vals, k_scaler, v_scaler, mask):
    """Per-shard attention for one kv-head slice.

    q:    [B, G, S, HD] f32 (rope'd queries for this core's head group)
    keys: [B, L, HD] f32 (dequant-count int8 values for this core's kv head)
    vals: [B, L, HD] f32
    k_scaler/v_scaler: [B, L] f32 (updated)
    mask: [B, 1, S, L] f32 additive
    returns out [B, G, S, HD] f32
    """
    out = np.empty((B, G, S, HD), dtype=np.float32)
    for bi in range(B):
        qb = q[bi].reshape(G * S, HD)  # [64, HD]
        scores = (qb @ keys[bi].T) * np.float32(HD**-0.5)  # [64, L]
        scores = scores * k_scaler[bi][None, :]
        scores = scores.reshape(G, S, -1) + mask[bi]  # [G,S,L] + [1,S,L]
        probs = _softmax(scores.reshape(G * S, -1).astype(np.float32), axis=-1)
        probs = probs * v_scaler[bi][None, :]
        out[bi] = (probs @ vals[bi]).reshape(G, S, HD)
    return out


def kernel(
    x,
    freqs_cos,
    freqs_sin,
    mask,
    cache_k,
    cache_v,
    k_scaler,
    v_scaler,
    wqkv_w,
    wqkv_s,
    wo_w,
    wo_s,
    input_pos,
):
    x = np.asarray(x, dtype=np.float32)
    freqs_cos = np.asarray(freqs_cos, dtype=np.float32)
    freqs_sin = np.asarray(freqs_sin, dtype=np.float32)
    mask = np.asarray(mask, dtype=np.float32)
    k_scaler = np.asarray(k_scaler, dtype=np.float32).copy()
    v_scaler = np.asarray(v_scaler, dtype=np.float32).copy()
    wqkv_s = np.asarray(wqkv_s, dtype=np.float32)
    wo_s = np.asarray(wo_s, dtype=np.float32)
    P = int(input_pos)

    # int8-valued tensors may arrive as int8 or int32 containers
    wqkv_f = np.asarray(wqkv_w).astype(np.float32)
    wo_f = np.asarray(wo_w).astype(np.float32)

    b, s, _ = x.shape

    # ---- fused int8 weight-only QKV projection ----
    qkv = (x.reshape(b * s, D) @ wqkv_f.T) * wqkv_s
    qkv = qkv.astype(np.float32).reshape(b, s, Q_SIZE + 2 * KV_SIZE)
    xq = qkv[..., :Q_SIZE].reshape(b, s, H, HD)
    xk = qkv[..., Q_SIZE : Q_SIZE + KV_SIZE].reshape(b, s, HKV, HD)
    xv = qkv[..., Q_SIZE + KV_SIZE :].reshape(b, s, HKV, HD)
    xq = _rope(xq, freqs_cos, freqs_sin)
    xk = _rope(xk, freqs_cos, freqs_sin)
    xk = xk.transpose(0, 2, 1, 3)  # [B, HKV, S, HD]
    xv = xv.transpose(0, 2, 1, 3)

    # ---- per-token int8 quantization of new K/V (global across kv heads) ----
    k_sc = (np.max(np.abs(xk), axis=(1, 3)) / 127.0 + 1e-8).astype(np.float32)
    v_sc = (np.max(np.abs(xv), axis=(1, 3)) / 127.0 + 1e-8).astype(np.float32)
    k_q = np.round(xk / k_sc[:, None, :, None]).astype(np.int8)
    v_q = np.round(xv / v_sc[:, None, :, None]).astype(np.int8)
    k_scaler[:, P : P + s] = k_sc
    v_scaler[:, P : P + s] = v_sc

    # ---- sharded attention over kv heads (1 kv head / core) ----
    # queries grouped: [B, HKV, G, S, HD]
    q_g = xq.transpose(0, 2, 1, 3).reshape(b, HKV, G, s, HD).astype(np.float32)

    cache_k = np.asarray(cache_k)
    cache_v = np.asarray(cache_v)
    # per-core key/value shards, updated with the freshly quantized chunk
    keys_all = cache_k.astype(np.float32).transpose(1, 0, 2, 3)  # [HKV, B, L, HD]
    vals_all = cache_v.astype(np.float32).transpose(1, 0, 2, 3)
    keys_all[:, :, P : P + s, :] = k_q.transpose(1, 0, 2, 3).astype(np.float32)
    vals_all[:, :, P : P + s, :] = v_q.transpose(1, 0, 2, 3).astype(np.float32)
    q_shard = q_g.transpose(1, 0, 2, 3, 4).copy()  # [HKV, B, G, S, HD]
    # row-parallel wo: core h takes columns of attn for heads [h*G,(h+1)*G)
    # wo_w is [D_out, H*HD]; slice contraction rows h*G*HD:(h+1)*G*HD
    wo_shard = (
        wo_f.T.reshape(HKV, G * HD, D).copy()
    )  # [HKV, G*HD, D] so partial = attn_slice @ wo_shard[h]

    try:
        out = _device_attention(
            q_shard, keys_all, vals_all, k_scaler, v_scaler, mask, wo_shard
        )
    except Exception:
        out = _host_attention(
            q_shard, keys_all, vals_all, k_scaler, v_scaler, mask, wo_shard
        )

    return (out * wo_s).astype(np.float32).reshape(b, s, D)


def _host_attention(q_shard, keys_all, vals_all, k_scaler, v_scaler, mask, wo_shard):
    acc = np.zeros((B * S, D), dtype=np.float32)
    for h in range(HKV):
        o = _core_attention(
            q_shard[h], keys_all[h], vals_all[h], k_scaler, v_scaler, mask
        )  # [B, G, S, HD]
        attn_slice = o.transpose(0, 2, 1, 3).reshape(B * S, G * HD)
        acc += attn_slice @ wo_shard[h]
    return acc


_PMAP_FN = None


def _get_pmap_fn():
    global _PMAP_FN
    if _PMAP_FN is not None:
        return _PMAP_FN
    import jax
    import jax.numpy as jnp
    from functools import partial

    devs = jax.devices()[:N_CORES]
    assert len(devs) == N_CORES

    @partial(jax.pmap, axis_name="c", devices=devs)
    def fn(q, keys, vals, ks, vs, mask, wo):
        # q [B,G,S,HD], keys/vals [B,L,HD], ks/vs [B,L], mask [B,1,S,L], wo [G*HD, D]
        scores = jnp.einsum("bgsd,bld->bgsl", q, keys) * (HD**-0.5)
        scores = scores * ks[:, None, None, :]
        scores = scores + mask
        probs = jax.nn.softmax(scores.astype(jnp.float32), axis=-1)
        probs = probs * vs[:, None, None, :]
        o = jnp.einsum("bgsl,bld->bgsd", probs, vals)  # [B,G,S,HD]
        attn_slice = o.transpose(0, 2, 1, 3).reshape(B * S, G * HD)
        partial_out = attn_slice @ wo  # [B*S, D]
        return jax.lax.psum(partial_out, "c")

    _PMAP_FN = fn
    return fn


def _device_attention(q_shard, keys_all, vals_all, k_scaler, v_scaler, mask, wo_shard):
    fn = _get_pmap_fn()
    ks_r = np.broadcast_to(k_scaler, (N_CORES,) + k_scaler.shape)
    vs_r = np.broadcast_to(v_scaler, (N_CORES,) + v_scaler.shape)
    mask_r = np.broadcast_to(mask, (N_CORES,) + mask.shape)
    out = fn(q_shard, keys_all, vals_all, ks_r, vs_r, mask_r, wo_shard)
    return np.asarray(out[0], dtype=np.float32)
